# revision 8
# baseline (speedup 1.0000x reference)
"""Trainium2 Bass kernel for nn_AffinityBiFC.

Reference computation (B=4, N=M=128, D=256, BD=1024):
    t  = einsum('bnd,dek->bnek', X, A)
    bi = einsum('bnek,bme->bnmk', t, Y)
    S  = einsum('bnmk,ok->bnmo', bi, W) + b        -> S[..., 0]  [B, N, M]

Algebraic collapse (exact reassociation):
    Aw[d, e] = sum_k A[d, e, k] * W[0, k]          # one streaming pass over A (268 MB)
    S[b]     = X[b] @ Aw @ Y[b].T + b              # tiny matmuls

Sharding: A is split over its first (d) axis across the 8 cores (each core
streams a contiguous 33.5 MB block and produces 32 rows of Aw).  Partial Aw
rows are AllGathered in three fp16 slices — the first two fire mid-stream and
are fully hidden; a tiny warm-up collective at kernel start absorbs the ncfw
cold-start latency so all real collectives run warm.  Every core then
redundantly computes the final small matmuls in fp16 (fp32 accumulate) and
writes the full output; the host takes core 0's copy and adds the bias.

Per-core pipeline:
  - DMA A in [128, dd, 2, 1024] tiles (partition = e%128); first groups are
    small so the DVE stream starts early, later groups are 4 MB.
  - DVE tensor_tensor mult (A_tile * W_rep); ACT activation(Copy, accum_out)
    sums over k -> acc[e%128, ec, dl].  DVE ~78us + ACT ~98us sit just under
    the ~100us DMA stream (DMA-bound at ~330-350 GB/s).
  - After 12 / 24 / 32 d-rows: PE-transpose the acc slice, cast fp16,
    AllGather it.  Slices interleave d rows core-major ("comb" order); the
    final matmuls contract over d in the same comb order (the host uploads
    X^T already permuted to match, and Y^T in natural order, both fp16).
"""

import numpy as np

B, N, D, KD = 4, 128, 256, 1024
P = 128
C = 8                   # cores
DL = D // C             # 32 d-rows per core
GROUPS = [1, 1, 2, 4, 4, 4, 4, 4, 4, 4]    # d-rows per DMA
SPLITS = [(0, 12), (12, 24), (24, 32)]      # collective slices (d-rows)
FLUSH_AT = {12: 0, 24: 1, 32: 2}
assert sum(GROUPS) == DL

_cached = {}


def _build_program():
    import concourse.bass as bass
    import concourse.mybir as mybir
    import concourse.tile as tile
    from concourse import bacc
    from concourse.masks import make_identity

    fp32 = mybir.dt.float32
    fp16 = mybir.dt.float16

    nc = bacc.Bacc(
        "TRN2",
        target_bir_lowering=False,
        debug=False,
        num_devices=C,
    )

    a_sh = nc.dram_tensor("a_sh", [DL, D, KD], fp32, kind="ExternalInput").ap()
    # host-staged: X^T fp16 in comb-split order [d', b, n], Y^T fp16 [e, b, m]
    xt_in = nc.dram_tensor("xt_in", [D, B, N], fp16, kind="ExternalInput").ap()
    yt_in = nc.dram_tensor("yt_in", [D, B, N], fp16, kind="ExternalInput").ap()
    w_rep = nc.dram_tensor("w_rep", [P, KD], fp32, kind="ExternalInput").ap()
    out = nc.dram_tensor("out", [B, N, N], fp32, kind="ExternalOutput").ap()

    with tile.TileContext(nc) as tc:
        with (
            tc.tile_pool(name="apool", bufs=4) as apool,
            tc.tile_pool(name="ppool", bufs=3) as ppool,
            tc.tile_pool(name="sbuf", bufs=1) as sbuf,
            tc.tile_pool(name="psum", bufs=4, space="PSUM") as psum,
            tc.tile_pool(name="dram", bufs=1, space="DRAM") as dram,
        ):
            # W first: the stream needs it immediately.
            w_sb = sbuf.tile([P, KD], fp32)
            nc.sync.dma_start(w_sb[:], w_rep[:])

            # warm-up collective: absorbs the ~11.5us ncfw cold-start so the
            # real AllGathers run with ~1us trigger latency.
            warm_in = dram.tile([1, 16], fp16)
            warm_out = dram.tile([C, 16], fp16, addr_space="Shared")
            warm_sb = sbuf.tile([1, 16], fp16)
            nc.vector.memset(warm_sb[:], 0.0)
            nc.sync.dma_start(warm_in[:], warm_sb[:])
            nc.gpsimd.collective_compute(
                "AllGather",
                mybir.AluOpType.bypass,
                replica_groups=[list(range(C))],
                ins=[warm_in.opt()],
                outs=[warm_out.opt()],
            )

            # acc[e_lo, ec, dl] = Aw[c*DL + dl, ec*128 + e_lo]
            acc = sbuf.tile([P, 2, DL], fp32)
            scratch = sbuf.tile([P, KD], fp32)

            ident = sbuf.tile([P, P], fp32)
            make_identity(nc, ident)

            awT = [sbuf.tile([hi - lo, D], fp16, name=f"awT{s}") for s, (lo, hi) in enumerate(SPLITS)]
            cc_in = [dram.tile([hi - lo, D], fp16, name=f"cc_in{s}") for s, (lo, hi) in enumerate(SPLITS)]
            cc_out = [
                dram.tile([C * (hi - lo), D], fp16, addr_space="Shared", name=f"cc_out{s}")
                for s, (lo, hi) in enumerate(SPLITS)
            ]

            def flush(s):
                lo, hi = SPLITS[s]
                for ec in range(2):
                    psa = psum.tile([P, P], fp32, tag="ps", name=f"psa{s}{ec}")
                    nc.tensor.transpose(psa[: hi - lo, :], acc[:, ec, lo:hi], ident)
                    nc.vector.tensor_copy(
                        out=awT[s][:, ec * P : (ec + 1) * P], in_=psa[: hi - lo, :]
                    )
                nc.sync.dma_start(cc_in[s][:], awT[s][:])
                nc.gpsimd.collective_compute(
                    "AllGather",
                    mybir.AluOpType.bypass,
                    replica_groups=[list(range(C))],
                    ins=[cc_in[s].opt()],
                    outs=[cc_out[s].opt()],
                )

            # main stream: A groups (all triggers early in program order)
            a_flat = a_sh.rearrange("dl (ec p) k -> p dl ec k", p=P)
            dl0 = 0
            for g, dd in enumerate(GROUPS):
                at = apool.tile([P, 4, 2, KD], fp32, tag="a", name=f"at{g}")
                nc.sync.dma_start(at[:, :dd, :, :], a_flat[:, dl0 : dl0 + dd, :, :])
                for j in range(dd):
                    dl = dl0 + j
                    for ec in range(2):
                        prod = ppool.tile([P, KD], fp32, tag="prod", name=f"pr{dl}{ec}")
                        nc.vector.tensor_tensor(
                            out=prod[:],
                            in0=at[:, j, ec, :],
                            in1=w_sb,
                            op=mybir.AluOpType.mult,
                        )
                        nc.scalar.activation(
                            out=scratch[:],
                            in_=prod[:],
                            func=mybir.ActivationFunctionType.Copy,
                            accum_out=acc[:, ec, dl : dl + 1],
                        )
                dl0 += dd
                if dl0 in FLUSH_AT:
                    flush(FLUSH_AT[dl0])

            # X^T comb tiles (partition dim = comb rows of each split) and Y^T
            xt = [
                sbuf.tile([C * (hi - lo), B, N], fp16, name=f"xt{s}")
                for s, (lo, hi) in enumerate(SPLITS)
            ]
            ofs = 0
            for s, (lo, hi) in enumerate(SPLITS):
                rows = C * (hi - lo)
                nc.sync.dma_start(xt[s][:], xt_in[ofs : ofs + rows])
                ofs += rows
            yT = sbuf.tile([P, 2, B, N], fp16)  # [e_lo, ec, b, m]
            nc.sync.dma_start(yT[:], yt_in.rearrange("(ec p) b m -> p ec b m", p=P))

            gsb = [
                sbuf.tile([C * (hi - lo), D], fp16, name=f"gsb{s}")
                for s, (lo, hi) in enumerate(SPLITS)
            ]
            for s in range(len(SPLITS)):
                nc.sync.dma_start(gsb[s][:], cc_out[s][:])

            # final matmuls: T^T[b][ec] = sum_splits Aw^T x X^T, then S[b]
            tT = sbuf.tile([P, 2, B, P], fp16)  # [e_lo, ec, b, n]
            s_sb = sbuf.tile([P, B, N], fp32)   # [n, b, m]
            nsplit = len(SPLITS)
            for b in range(B):
                for ec in range(2):
                    psT = psum.tile([P, P], fp32, tag="ps", name=f"psT{b}{ec}")
                    for s in range(nsplit):
                        nc.tensor.matmul(
                            psT,
                            lhsT=gsb[s][:, ec * P : (ec + 1) * P],
                            rhs=xt[s][:, b, :],
                            start=(s == 0),
                            stop=(s == nsplit - 1),
                        )
                    nc.any.tensor_copy(out=tT[:, ec, b, :], in_=psT)
                psS = psum.tile([P, P], fp32, tag="ps", name=f"psS{b}")
                for ec in range(2):
                    nc.tensor.matmul(
                        psS,
                        lhsT=tT[:, ec, b, :],
                        rhs=yT[:, ec, b, :],
                        start=(ec == 0),
                        stop=(ec == 1),
                    )
                nc.any.tensor_copy(out=s_sb[:, b, :], in_=psS)

            nc.sync.dma_start(out.rearrange("b n m -> n b m"), s_sb[:])

    nc.compile()
    return nc


def _get_program():
    if "nc" not in _cached:
        _cached["nc"] = _build_program()
    return _cached["nc"]


def _prep_xt(X):
    """X^T in comb-split order: rows grouped by split, then core-major.

    Row index within split s (rows [lo,hi)): r = c*(hi-lo) + (dl-lo)
    maps to d = c*DL + dl.  Matches the AllGather concatenation order.
    """
    Xt = np.ascontiguousarray(X.transpose(2, 0, 1), dtype=np.float16)  # [d, b, n]
    order = []
    for lo, hi in SPLITS:
        for c in range(C):
            for dl in range(lo, hi):
                order.append(c * DL + dl)
    return np.ascontiguousarray(Xt[np.array(order)])


def _run(X, Y, A, W, b, trace=False, **trace_kwargs):
    from concourse.bass_utils import run_bass_kernel_spmd

    nc = _get_program()

    A = np.ascontiguousarray(A, dtype=np.float32)
    W = np.ascontiguousarray(W, dtype=np.float32)
    xt = _prep_xt(np.asarray(X, dtype=np.float32))
    yt = np.ascontiguousarray(
        np.asarray(Y, dtype=np.float32).transpose(2, 0, 1), dtype=np.float16
    )
    w_rep = np.ascontiguousarray(
        np.broadcast_to(W.reshape(1, KD), (P, KD)), dtype=np.float32
    )

    core_ids = list(range(C))
    in_maps = [
        {
            "a_sh": A[c * DL : (c + 1) * DL],
            "xt_in": xt,
            "yt_in": yt,
            "w_rep": w_rep,
        }
        for c in core_ids
    ]

    res = run_bass_kernel_spmd(nc, in_maps, core_ids, trace=trace, **trace_kwargs)
    out = np.asarray(res.results[0]["out"], dtype=np.float32)
    out = out + np.float32(b.reshape(-1)[0])
    return out, res


def kernel(X, Y, A, W, b):
    out, _ = _run(X, Y, A, W, b, trace=False)
    return out
